# revision 22
# baseline (speedup 1.0000x reference)
"""AdaPool2d forward kernel for Trainium2 (8 NeuronCores, data-parallel).

x: [16, 64, 224, 224] f32, beta: [112, 112] f32 (clamped to [0,1]).
K=2 pooling, stride 2 -> out [16, 64, 112, 112].

out = beta * EDSCW + (1-beta) * EM where
  EDSCW = softmax-over-taps(dice(t, avg)) . taps
  EM    = softmax-over-taps(taps) . taps         (SoftPool)

Sharding: batch across 8 cores (2 batches/core); each core's 2*64 = 128
(b,c)-planes map onto the 128 SBUF partitions; taps packed [P, 4, NWIN].

Key structural ideas vs a naive port:
 1. beta = clip(randn,0,1) is EXACTLY 0 for ~half the window positions
    and EXACTLY 1 for ~16%. The window axis is freely permutable, so the
    host sorts window positions into three classes:
      class 0 (beta==0): out = EM       -> skip the whole EDSCW branch
      class 1 (beta==1): out = EDSCW    -> skip the whole EM branch
      mixed  (0<beta<1): full blend
    and un-permutes the result. This removes ~40% of device work with
    bit-exact semantics.
 2. dsc is computed by ONE fused custom DVE op (DSCS) from (t, a=s/4):
      dsc = 2*t*a/(t^2+a^2+eps)
    using the BITWISE_NOT-seed + 1 Chebyshev-tuned Newton step for the
    reciprocal, restructured as p*c1 - p*(D*y0) (p = 2ta*y0) to fit the
    8-stage DVE datapath. No separate r = t/avg pass is needed.
 3. exp() passes run on the Scalar engine (table exp); the per-window
    sums run as identity-matmul trees on the (otherwise idle) PE; the
    EM tap product f*t runs on Pool for class-0 chunks (Pool is idle
    there) and on DVE elsewhere; divisions are custom-NR DVE ops fed
    straight from PSUM.
"""

import sys
import os
import numpy as np

for _p in ("/opt/trn_rl_repo", "/root/.axon_site/_ro/trn_rl_repo"):
    if os.path.isdir(_p) and _p not in sys.path:
        sys.path.insert(0, _p)

B, C, H, W = 16, 64, 224, 224
OH, OW = 112, 112
NWIN = OH * OW          # 12544 windows per plane
NCORES = 8
BPC = B // NCORES       # batches per core
P = BPC * C             # 128 planes per core == SBUF partitions

_COMPILED = {}

# Chebyshev-tuned 1-step Newton-from-BITWISE_NOT-seed reciprocal consts
_CH_S0 = -0.23549792
_CH_S1 = 2.0017324


def _install_op(dvo, op):
    dvo.OPS.append(op)
    dvo.CUSTOM_DVE_SPECS[op.name] = op.spec
    dvo._SUB_OPCODE_FOR_NAME[op.name] = dvo._CUSTOM_DVE_ROW_BASE + len(dvo.OPS) - 1
    assert max(dvo._SUB_OPCODE_FOR_NAME.values()) < 0x20


def _register(name, spec_fn):
    from concourse import dve_ops as dvo
    from concourse.dve_spec import lower as dve_lower, _has_src1
    from concourse.dve_uop import DveOpSpec

    for op in dvo.OPS:
        if op.name == name:
            return op
    spec = spec_fn()
    shas = {}
    for ver in ("v3", "v4"):
        try:
            tmp = DveOpSpec(
                name=name, opcode=0, uops=dve_lower(spec, ver=ver),
                rd1_en=_has_src1(spec),
            )
            shas[ver] = tmp.sha(ver)
        except Exception:
            pass
    op = dvo.DveOp(name, spec, False, shas)
    _install_op(dvo, op)
    return op


def _register_dsc_op():
    """DSC1B: out = 2*Src0 * nr1(Src0^2 + 1)  ~=  2r/(r^2+1), 1-Newton-step
    reciprocal from the BITWISE_NOT exponent-flip seed. The DVE pipeline has
    8 sequential ALU slots; this body uses exactly 8 (the *2 rides the
    downstream Exp activation's scale)."""
    from concourse.dve_spec import (
        Spec, Src0, One, Bin, AluOp, C0, C1, sq,
    )

    def mk():
        x = sq(Src0) + One
        nx = Bin(AluOp.BITWISE_NOT, x, x)
        y0 = nx * C0
        y1 = y0 * (C1 - x * y0)
        body = y1 * Src0

        def _ref(in0, in1, c0, c1, c2):
            x = (in0.astype(np.float32) ** 2 + 1.0).astype(np.float32)
            nx = (~x.view(np.int32)).view(np.float32)
            y0 = nx * np.float32(c0)
            y1 = y0 * (np.float32(c1) - x * y0)
            return y1 * in0.astype(np.float32)

        return Spec(body=body, reference=_ref)

    return _register("DSC1B_ANT", mk)


def _register_recip_avg_op():
    """RECIPAVG_ANT: out = nr1(Src0*C2 + c3) ~= 1/(s*0.25 + eps), one
    Chebyshev-tuned Newton step from the BITWISE_NOT seed. c3 (eps) rides
    the spilled-C3 slot, passed as a [P,1] AP via in1."""
    from concourse.dve_spec import (
        Spec, Src0, Bin, AluOp, C0, C1, C2, C3, _spill_c3_to_src1,
    )

    def mk():
        x = Src0 * C2 + C3
        nx = Bin(AluOp.BITWISE_NOT, x, x)
        y0 = nx * C0
        body = _spill_c3_to_src1(y0 * (C1 - x * y0))

        def _ref(in0, in1, c0, c1, c2):
            x = (in0.astype(np.float32) * np.float32(c2)
                 + np.asarray(in1, np.float32).reshape(-1, 1)).astype(
                     np.float32)
            nx = (~x.view(np.int32)).view(np.float32)
            y0 = nx * np.float32(c0)
            return y0 * (np.float32(c1) - x * y0)

        return Spec(body=body, reference=_ref)

    return _register("RECIPAVG_ANT", mk)


def _register_div_op():
    """DIV1NR_ANT: out = Src0 * nr1(Src1) ~= Src0/Src1 at ~0.2% max rel err
    (BITWISE_NOT seed + one Chebyshev-tuned Newton step)."""
    from concourse.dve_spec import (
        Spec, Src0, Src1, Bin, AluOp, C0, C1,
    )

    def mk():
        nx = Bin(AluOp.BITWISE_NOT, Src1, Src1)
        y0 = nx * C0
        y1 = y0 * (C1 - Src1 * y0)
        body = y1 * Src0

        def _ref(in0, in1, c0, c1, c2):
            x = in1.astype(np.float32)
            nx = (~x.view(np.int32)).view(np.float32)
            y0 = nx * np.float32(c0)
            y1 = y0 * (np.float32(c1) - x * y0)
            return y1 * in0.astype(np.float32)

        return Spec(body=body, reference=_ref)

    return _register("DIV1NR_ANT", mk)


def _chunk_sizes(total, first, step=512):
    """Chunk a phase's window count: a small first chunk (pipeline fill),
    then full steps."""
    sizes = []
    if total <= 0:
        return sizes
    f = min(first, total)
    sizes.append(f)
    r = total - f
    while r > 0:
        s = min(step, r)
        sizes.append(s)
        r -= s
    return sizes


def _build(n0, n1, nm):
    import concourse.bacc as bacc
    import concourse.mybir as mybir
    from concourse.tile import TileContext

    bf16 = mybir.dt.bfloat16
    f32 = mybir.dt.float32
    Exp = mybir.ActivationFunctionType.Exp

    dsc_op = _register_dsc_op()
    div_op = _register_div_op()
    ravg_op = _register_recip_avg_op()

    nc = bacc.Bacc()
    x4 = nc.declare_dram_parameter("x4", [P, 4, NWIN], bf16, isOutput=False)
    ident_d = nc.declare_dram_parameter("ident", [P, P], bf16, isOutput=False)
    out_d = nc.declare_dram_parameter("out", [P, NWIN], bf16, isOutput=True)
    if nm > 0:
        betam_d = nc.declare_dram_parameter("betam", [P, nm], bf16,
                                            isOutput=False)

    # (phase, offset, n) chunk lists per class (window-permuted DRAM order:
    # [em | ed | mx]); emission order interleaves light em/ed chunks between
    # heavy mx chunks so stalled engines always have filler work.
    em_chunks, ed_chunks, mx_chunks = [], [], []
    o = 0
    for sz in _chunk_sizes(n0, 256):
        em_chunks.append(("em", o, sz))
        o += sz
    for sz in _chunk_sizes(n1, 256 if n0 == 0 else 512):
        ed_chunks.append(("ed", o, sz))
        o += sz
    for sz in _chunk_sizes(nm, 256 if (n0 == 0 and n1 == 0) else 512):
        mx_chunks.append(("mx", o, sz))
        o += sz
    assert o == NWIN
    light = em_chunks + ed_chunks
    chunks = []
    li, mi = 0, 0
    # ~ratio light:mx interleave
    nl, nmx = len(light), len(mx_chunks)
    while li < nl or mi < nmx:
        take_l = (li * nmx <= mi * nl) if nmx else True
        if li < nl and (take_l or mi >= nmx):
            chunks.append(light[li])
            li += 1
        elif mi < nmx:
            chunks.append(mx_chunks[mi])
            mi += 1

    with TileContext(nc) as tc:
        with tc.tile_pool(name="pool", bufs=2) as pool, \
             tc.tile_pool(name="psum", bufs=1, space="PSUM") as psum:
            ident = pool.tile([P, P], bf16, tag="ident", name="ident", bufs=1)
            nc.sync.dma_start(out=ident[:, :], in_=ident_d[:, :])
            epsc = pool.tile([P, 1], f32, tag="epsc", name="epsc", bufs=1)
            nc.gpsimd.memset(epsc[:, :], 1e-12)
            # dummy activation: pull the exp table load off the first
            # chunk's critical path (overlaps the input DMA)
            warm = pool.tile([P, 8], bf16, tag="warm", name="warm", bufs=1)
            nc.gpsimd.memset(warm[:, :], 0.0)
            nc.scalar.activation(warm[:, :], warm[:, :], Exp)

            mo = 0
            for ci, (ph, o, n) in enumerate(chunks):
                sl = slice(o, o + n)

                def T(tag, bufs=2, shape=None, dt=bf16):
                    return pool.tile(shape or [P, n], dt, tag=tag, name=tag,
                                     bufs=bufs)

                def T4(tag, bufs=2):
                    return pool.tile([P, 4, n], bf16, tag=tag, name=tag,
                                     bufs=bufs)

                def tree(src4, tag, bufs):
                    ps = psum.tile([P, n], f32, tag=tag, name=tag, bufs=bufs)
                    for i in range(4):
                        nc.tensor.matmul(ps[:, :], ident[:, :],
                                         src4[:, i, :],
                                         start=(i == 0), stop=(i == 3))
                    return ps

                with tc.high_priority(offset=30):
                    x4t = T4("x4t", bufs=4)
                    nc.sync.dma_start(out=x4t[:, :, :], in_=x4[:, :, sl])

                # ---------- EM branch (phases em, mx) ----------
                def em_branch(pf_engine):
                    f_all = T4("f_all")
                    nc.scalar.activation(f_all[:, :, :], x4t[:, :, :], Exp)
                    pf_all = T4("pf_all")
                    if pf_engine == "pool":
                        for hh in range(2):
                            i0, i1 = 2 * hh, 2 * hh + 2
                            nc.gpsimd.tensor_mul(pf_all[:, i0:i1, :],
                                                 f_all[:, i0:i1, :],
                                                 x4t[:, i0:i1, :])
                    else:
                        nc.vector.tensor_mul(pf_all[:, :, :], f_all[:, :, :],
                                             x4t[:, :, :])
                    F_ps = tree(f_all, "ps_A" if ph == "em" else "ps_C",
                                bufs=2 if ph == "em" else 1)
                    Qn_ps = tree(pf_all, "ps_B" if ph == "em" else "ps_D",
                                 bufs=3 if ph == "em" else 1)
                    # denominator must leave PSUM (1-PSUM-operand limit);
                    # Pool is idle in em-phase, Act does it in mixed
                    F_sb = T("F_sb", bufs=3)
                    with tc.high_priority(offset=15):
                        nc.scalar.copy(F_sb[:, :], F_ps[:, :])
                        em = T("em", bufs=3)
                        nc.vector._custom_dve(
                            div_op, out=em[:, :], in0=Qn_ps[:, :],
                            in1=F_sb[:, :], s0=_CH_S0, s1=_CH_S1,
                        )
                    return em

                # ---------- EDSCW branch (phases ed, mx) ----------
                def ed_branch(pe_engine="dve"):
                    with tc.high_priority(offset=25):
                        s_ps = psum.tile([P, n], f32, tag="ps_s", name="ps_s",
                                         bufs=1)
                        for i in range(4):
                            nc.tensor.matmul(s_ps[:, :], ident[:, :],
                                             x4t[:, i, :],
                                             start=(i == 0), stop=(i == 3))
                        # inva = 1/(s/4 + 1e-12) fused, straight from PSUM
                        inva = T("inva", bufs=3)
                        nc.vector._custom_dve(
                            ravg_op, out=inva[:, :], in0=s_ps[:, :],
                            in1=epsc[:, :], s0=_CH_S0, s1=_CH_S1, imm2=0.25,
                        )
                        r_all = T4("r_all", bufs=2)
                        inva_b = inva[:, :].unsqueeze(1).broadcast_to(
                            [P, 4, n])
                        nc.vector.tensor_mul(r_all[:, :, :], x4t[:, :, :],
                                             inva_b)
                        dsc_all = T4("dsc_all", bufs=2)
                        nc.vector._custom_dve(
                            dsc_op, out=dsc_all[:, :, :], in0=r_all[:, :, :],
                            s0=_CH_S0, s1=_CH_S1,
                        )
                    e_all = T4("e_all")
                    nc.scalar.activation(e_all[:, :, :], dsc_all[:, :, :],
                                         Exp, scale=2.0)
                    pe_all = T4("pe_all")
                    if pe_engine == "pool":
                        for hh in range(2):
                            i0, i1 = 2 * hh, 2 * hh + 2
                            nc.gpsimd.tensor_mul(pe_all[:, i0:i1, :],
                                                 e_all[:, i0:i1, :],
                                                 x4t[:, i0:i1, :])
                    else:
                        nc.vector.tensor_mul(pe_all[:, :, :], e_all[:, :, :],
                                             x4t[:, :, :])
                    E_ps = tree(e_all, "ps_A", bufs=2)
                    Pn_ps = tree(pe_all, "ps_B", bufs=3)
                    E_sb = T("E_sb", bufs=3)
                    ed = T("ed", bufs=3)
                    with tc.high_priority(offset=15):
                        nc.scalar.copy(E_sb[:, :], E_ps[:, :])
                        nc.vector._custom_dve(
                            div_op, out=ed[:, :], in0=Pn_ps[:, :],
                            in1=E_sb[:, :], s0=_CH_S0, s1=_CH_S1,
                        )
                    return ed

                if ph == "em":
                    # em-phase is DMA-bound-ish; alternate pf between DVE
                    # and the otherwise-idle Pool
                    em = em_branch("pool" if ci % 2 == 0 else "dve")
                    nc.sync.dma_start(out=out_d[:, sl], in_=em[:, :])
                elif ph == "ed":
                    ed = ed_branch(pe_engine="pool")
                    nc.sync.dma_start(out=out_d[:, sl], in_=ed[:, :])
                else:
                    # f-exp is ready as soon as x4t lands: emit EM first so
                    # Act/Pool start early; the EDSCW head chain is pulled
                    # forward by its priorities anyway
                    em = em_branch("pool")
                    ed = ed_branch(pe_engine="dve")
                    bb = T("bb", bufs=2)
                    nc.sync.dma_start(out=bb[:, :], in_=betam_d[:, mo:mo + n])
                    # out = em + bb*(ed - em): all on Pool (SBUF-only ops)
                    dif = T("dif", bufs=3)
                    nc.gpsimd.tensor_sub(dif[:, :], ed[:, :], em[:, :])
                    bd = T("bd", bufs=3)
                    nc.gpsimd.tensor_mul(bd[:, :], dif[:, :], bb[:, :])
                    ot = T("ot", bufs=3)
                    nc.gpsimd.tensor_add(ot[:, :], em[:, :], bd[:, :])
                    nc.sync.dma_start(out=out_d[:, sl], in_=ot[:, :])
                    mo += n
    nc.finalize()
    return nc


def _get_nc(n0, n1, nm):
    key = (n0, n1, nm)
    if key not in _COMPILED:
        _COMPILED[key] = _build(*key)
    return _COMPILED[key]


def _shard_inputs(x, beta):
    """Host-side: classify window positions by beta, permute windows into
    [beta==0 | beta==1 | mixed] order, split taps, pack [P,4,NWIN] bf16."""
    import ml_dtypes

    bfl = ml_dtypes.bfloat16
    x = np.ascontiguousarray(x, dtype=np.float32)
    beta = np.asarray(beta, dtype=np.float32).reshape(NWIN)

    cls0 = beta <= 0.0
    cls1 = beta >= 1.0
    clsm = ~(cls0 | cls1)
    perm = np.concatenate(
        [np.nonzero(cls0)[0], np.nonzero(cls1)[0], np.nonzero(clsm)[0]])
    n0, n1 = int(cls0.sum()), int(cls1.sum())
    nm = NWIN - n0 - n1
    inv = np.empty(NWIN, dtype=np.int64)
    inv[perm] = np.arange(NWIN)

    bm = beta[perm[n0 + n1:]].astype(bfl) if nm else None
    ident = np.ascontiguousarray(np.eye(P, dtype=bfl))

    in_maps = []
    for core in range(NCORES):
        planes = x[core * BPC:(core + 1) * BPC].reshape(P, H, W)
        v = planes.reshape(P, OH, 2, OW, 2)
        x4 = np.empty((P, 4, NWIN), dtype=bfl)
        x4[:, 0, :] = v[:, :, 0, :, 0].reshape(P, NWIN)[:, perm]
        x4[:, 1, :] = v[:, :, 0, :, 1].reshape(P, NWIN)[:, perm]
        x4[:, 2, :] = v[:, :, 1, :, 0].reshape(P, NWIN)[:, perm]
        x4[:, 3, :] = v[:, :, 1, :, 1].reshape(P, NWIN)[:, perm]
        m = {"x4": x4, "ident": ident}
        if nm:
            m["betam"] = np.ascontiguousarray(
                np.broadcast_to(bm.reshape(1, nm), (P, nm)))
        in_maps.append(m)
    return in_maps, (n0, n1, nm), inv


LAST = {}


def kernel(x, beta, trace=False, trace_kwargs=None):
    from concourse.bass_utils import run_bass_kernel_spmd

    in_maps, (n0, n1, nm), inv = _shard_inputs(np.asarray(x),
                                               np.asarray(beta))
    nc = _get_nc(n0, n1, nm)
    res = run_bass_kernel_spmd(
        nc, in_maps, core_ids=list(range(NCORES)),
        trace=trace, **(trace_kwargs or {}),
    )
    LAST["exec_time_ns"] = getattr(res, "exec_time_ns", None)
    LAST["results"] = res
    LAST["nc"] = nc
    out = np.empty((B, C, OH, OW), dtype=np.float32)
    for core in range(NCORES):
        o = np.asarray(res.results[core]["out"]).astype(np.float32)
        out[core * BPC:(core + 1) * BPC] = o[:, inv].reshape(BPC, C, OH, OW)
    return out


# revision 27
# speedup vs baseline: 1.0599x; 1.0599x over previous
"""AdaPool2d forward kernel for Trainium2 (8 NeuronCores, data-parallel).

x: [16, 64, 224, 224] f32, beta: [112, 112] f32 (clamped to [0,1]).
K=2 pooling, stride 2 -> out [16, 64, 112, 112].

out = beta * EDSCW + (1-beta) * EM where
  EDSCW = softmax-over-taps(dice(t, avg)) . taps
  EM    = softmax-over-taps(taps) . taps         (SoftPool)

Sharding: batch across 8 cores (2 batches/core); each core's 2*64 = 128
(b,c)-planes map onto the 128 SBUF partitions; taps packed [P, 4, NWIN].

Key structural ideas vs a naive port:
 1. beta = clip(randn,0,1) is EXACTLY 0 for ~half the window positions
    and EXACTLY 1 for ~16%. The window axis is freely permutable, so the
    host sorts window positions into three classes:
      class 0 (beta==0): out = EM       -> skip the whole EDSCW branch
      class 1 (beta==1): out = EDSCW    -> skip the whole EM branch
      mixed  (0<beta<1): full blend
    and un-permutes the result. This removes ~40% of device work with
    bit-exact semantics.
 2. dsc is computed by ONE fused custom DVE op (DSCS) from (t, a=s/4):
      dsc = 2*t*a/(t^2+a^2+eps)
    using the BITWISE_NOT-seed + 1 Chebyshev-tuned Newton step for the
    reciprocal, restructured as p*c1 - p*(D*y0) (p = 2ta*y0) to fit the
    8-stage DVE datapath. No separate r = t/avg pass is needed.
 3. exp() passes run on the Scalar engine (table exp); the per-window
    sums run as identity-matmul trees on the (otherwise idle) PE; the
    EM tap product f*t runs on Pool for class-0 chunks (Pool is idle
    there) and on DVE elsewhere; divisions are custom-NR DVE ops fed
    straight from PSUM.
"""

import sys
import os
import numpy as np

for _p in ("/opt/trn_rl_repo", "/root/.axon_site/_ro/trn_rl_repo"):
    if os.path.isdir(_p) and _p not in sys.path:
        sys.path.insert(0, _p)

B, C, H, W = 16, 64, 224, 224
OH, OW = 112, 112
NWIN = OH * OW          # 12544 windows per plane
NCORES = 8
BPC = B // NCORES       # batches per core
P = BPC * C             # 128 planes per core == SBUF partitions

_COMPILED = {}

# Chebyshev-tuned 1-step Newton-from-BITWISE_NOT-seed reciprocal consts
_CH_S0 = -0.23549792
_CH_S1 = 2.0017324


def _install_op(dvo, op):
    dvo.OPS.append(op)
    dvo.CUSTOM_DVE_SPECS[op.name] = op.spec
    dvo._SUB_OPCODE_FOR_NAME[op.name] = dvo._CUSTOM_DVE_ROW_BASE + len(dvo.OPS) - 1
    assert max(dvo._SUB_OPCODE_FOR_NAME.values()) < 0x20


def _register(name, spec_fn):
    from concourse import dve_ops as dvo
    from concourse.dve_spec import lower as dve_lower, _has_src1
    from concourse.dve_uop import DveOpSpec

    for op in dvo.OPS:
        if op.name == name:
            return op
    spec = spec_fn()
    shas = {}
    for ver in ("v3", "v4"):
        try:
            tmp = DveOpSpec(
                name=name, opcode=0, uops=dve_lower(spec, ver=ver),
                rd1_en=_has_src1(spec),
            )
            shas[ver] = tmp.sha(ver)
        except Exception:
            pass
    op = dvo.DveOp(name, spec, False, shas)
    _install_op(dvo, op)
    return op


def _register_dsc_op():
    """DSC1B: out = 2*Src0 * nr1(Src0^2 + 1)  ~=  2r/(r^2+1), 1-Newton-step
    reciprocal from the BITWISE_NOT exponent-flip seed. The DVE pipeline has
    8 sequential ALU slots; this body uses exactly 8 (the *2 rides the
    downstream Exp activation's scale)."""
    from concourse.dve_spec import (
        Spec, Src0, One, Bin, AluOp, C0, C1, sq,
    )

    def mk():
        x = sq(Src0) + One
        nx = Bin(AluOp.BITWISE_NOT, x, x)
        y0 = nx * C0
        y1 = y0 * (C1 - x * y0)
        body = y1 * Src0

        def _ref(in0, in1, c0, c1, c2):
            x = (in0.astype(np.float32) ** 2 + 1.0).astype(np.float32)
            nx = (~x.view(np.int32)).view(np.float32)
            y0 = nx * np.float32(c0)
            y1 = y0 * (np.float32(c1) - x * y0)
            return y1 * in0.astype(np.float32)

        return Spec(body=body, reference=_ref)

    return _register("DSC1B_ANT", mk)


def _register_recip_avg_op():
    """RECIPAVG_ANT: out = nr1(Src0*C2 + c3) ~= 1/(s*0.25 + eps), one
    Chebyshev-tuned Newton step from the BITWISE_NOT seed. c3 (eps) rides
    the spilled-C3 slot, passed as a [P,1] AP via in1."""
    from concourse.dve_spec import (
        Spec, Src0, Bin, AluOp, C0, C1, C2, C3, _spill_c3_to_src1,
    )

    def mk():
        x = Src0 * C2 + C3
        nx = Bin(AluOp.BITWISE_NOT, x, x)
        y0 = nx * C0
        body = _spill_c3_to_src1(y0 * (C1 - x * y0))

        def _ref(in0, in1, c0, c1, c2):
            x = (in0.astype(np.float32) * np.float32(c2)
                 + np.asarray(in1, np.float32).reshape(-1, 1)).astype(
                     np.float32)
            nx = (~x.view(np.int32)).view(np.float32)
            y0 = nx * np.float32(c0)
            return y0 * (np.float32(c1) - x * y0)

        return Spec(body=body, reference=_ref)

    return _register("RECIPAVG_ANT", mk)


def _register_div_op():
    """DIV1NR_ANT: out = Src0 * nr1(Src1) ~= Src0/Src1 at ~0.2% max rel err
    (BITWISE_NOT seed + one Chebyshev-tuned Newton step)."""
    from concourse.dve_spec import (
        Spec, Src0, Src1, Bin, AluOp, C0, C1,
    )

    def mk():
        nx = Bin(AluOp.BITWISE_NOT, Src1, Src1)
        y0 = nx * C0
        y1 = y0 * (C1 - Src1 * y0)
        body = y1 * Src0

        def _ref(in0, in1, c0, c1, c2):
            x = in1.astype(np.float32)
            nx = (~x.view(np.int32)).view(np.float32)
            y0 = nx * np.float32(c0)
            y1 = y0 * (np.float32(c1) - x * y0)
            return y1 * in0.astype(np.float32)

        return Spec(body=body, reference=_ref)

    return _register("DIV1NR_ANT", mk)


def _chunk_sizes(total, first, step=512):
    """Chunk a phase's window count: a small first chunk (pipeline fill),
    then full steps."""
    sizes = []
    if total <= 0:
        return sizes
    f = min(first, total)
    sizes.append(f)
    r = total - f
    while r > 0:
        s = min(step, r)
        sizes.append(s)
        r -= s
    return sizes


def _build(n0, n1, nm):
    import concourse.bacc as bacc
    import concourse.mybir as mybir
    from concourse.tile import TileContext

    bf16 = mybir.dt.bfloat16
    f32 = mybir.dt.float32
    Exp = mybir.ActivationFunctionType.Exp

    dsc_op = _register_dsc_op()
    div_op = _register_div_op()
    ravg_op = _register_recip_avg_op()

    nc = bacc.Bacc()
    x4 = nc.declare_dram_parameter("x4", [P, 4, NWIN], bf16, isOutput=False)
    ident_d = nc.declare_dram_parameter("ident", [P, P], bf16, isOutput=False)
    out_d = nc.declare_dram_parameter("out", [P, NWIN], bf16, isOutput=True)
    if nm > 0:
        betam_d = nc.declare_dram_parameter("betam", [P, nm], bf16,
                                            isOutput=False)

    # (phase, offset, n) chunk lists per class (window-permuted DRAM order:
    # [em | ed | mx]); emission order interleaves light em/ed chunks between
    # heavy mx chunks so stalled engines always have filler work.
    em_chunks, ed_chunks, mx_chunks = [], [], []
    o = 0
    for sz in _chunk_sizes(n0, 256):
        em_chunks.append(("em", o, sz))
        o += sz
    for sz in _chunk_sizes(n1, 256 if n0 == 0 else 512):
        ed_chunks.append(("ed", o, sz))
        o += sz
    for sz in _chunk_sizes(nm, 256 if (n0 == 0 and n1 == 0) else 512):
        mx_chunks.append(("mx", o, sz))
        o += sz
    assert o == NWIN
    light = em_chunks + ed_chunks
    chunks = []
    li, mi = 0, 0
    # ~ratio light:mx interleave
    nl, nmx = len(light), len(mx_chunks)
    while li < nl or mi < nmx:
        take_l = (li * nmx <= mi * nl) if nmx else True
        if li < nl and (take_l or mi >= nmx):
            chunks.append(light[li])
            li += 1
        elif mi < nmx:
            chunks.append(mx_chunks[mi])
            mi += 1

    with TileContext(nc) as tc:
        with tc.tile_pool(name="pool", bufs=2) as pool, \
             tc.tile_pool(name="psum", bufs=1, space="PSUM") as psum:
            ident = pool.tile([P, P], bf16, tag="ident", name="ident", bufs=1)
            nc.sync.dma_start(out=ident[:, :], in_=ident_d[:, :])
            epsc = pool.tile([P, 1], f32, tag="epsc", name="epsc", bufs=1)
            nc.gpsimd.memset(epsc[:, :], 1e-12)
            # dummy activation: pull the exp table load off the first
            # chunk's critical path (overlaps the input DMA)
            warm = pool.tile([P, 8], bf16, tag="warm", name="warm", bufs=1)
            nc.gpsimd.memset(warm[:, :], 0.0)
            nc.scalar.activation(warm[:, :], warm[:, :], Exp)

            mo = 0
            # running projected engine-busy (ns); flexible ops go greedily
            # to the cheaper-loaded engine. Fixed work is added as emitted.
            load = {"dve": 0.0, "pool": 0.0}

            def tap_mul(out, a, b, tag_cost=1.0):
                """t*x product over [P,4,n]: DVE TT 2x vs Pool TT 0.42."""
                nf = out.shape[1] * out.shape[2]
                cd = nf * 0.52 + 120
                cp = nf * 1.984 + 95
                if load["dve"] + cd <= load["pool"] + cp:
                    load["dve"] += cd
                    nc.vector.tensor_mul(out[:, :, :], a[:, :, :], b[:, :, :])
                else:
                    load["pool"] += cp
                    for hh in range(2):
                        i0, i1 = 2 * hh, 2 * hh + 2
                        nc.gpsimd.tensor_mul(out[:, i0:i1, :],
                                             a[:, i0:i1, :], b[:, i0:i1, :])

            def win_op(fn_dve, fn_pool, nf):
                cd = nf * 0.52 + 120
                cp = nf * 0.833 / 0.42 + 95
                if load["dve"] + cd <= load["pool"] + cp:
                    load["dve"] += cd
                    fn_dve()
                else:
                    load["pool"] += cp
                    fn_pool()

            for ci, (ph, o, n) in enumerate(chunks):
                sl = slice(o, o + n)

                def T(tag, bufs=2, shape=None, dt=bf16):
                    return pool.tile(shape or [P, n], dt, tag=tag, name=tag,
                                     bufs=bufs)

                def T4(tag, bufs=2):
                    return pool.tile([P, 4, n], bf16, tag=tag, name=tag,
                                     bufs=bufs)

                def tree(src4, tag, bufs):
                    ps = psum.tile([P, n], f32, tag=tag, name=tag, bufs=bufs)
                    for i in range(4):
                        nc.tensor.matmul(ps[:, :], ident[:, :],
                                         src4[:, i, :],
                                         start=(i == 0), stop=(i == 3))
                    return ps

                with tc.high_priority(offset=30):
                    x4t = T4("x4t", bufs=4)
                    nc.sync.dma_start(out=x4t[:, :, :], in_=x4[:, :, sl])

                # ---------- EM branch (phases em, mx) ----------
                def em_branch():
                    f_all = T4("f_all", bufs=3)
                    nc.scalar.activation(f_all[:, :, :], x4t[:, :, :], Exp)
                    pf_all = T4("pf_all", bufs=3)
                    tap_mul(pf_all, f_all, x4t)
                    F_ps = tree(f_all, "ps_A" if ph == "em" else "ps_C",
                                bufs=2 if ph == "em" else 1)
                    Qn_ps = tree(pf_all, "ps_B" if ph == "em" else "ps_D",
                                 bufs=3 if ph == "em" else 1)
                    # denominator must leave PSUM (1-PSUM-operand limit);
                    # Pool is idle in em-phase, Act does it in mixed
                    F_sb = T("F_sb", bufs=3)
                    with tc.high_priority(offset=15):
                        nc.scalar.copy(F_sb[:, :], F_ps[:, :])
                        em = T("em", bufs=3)
                        load["dve"] += n * 1.04 + 250
                        nc.vector._custom_dve(
                            div_op, out=em[:, :], in0=Qn_ps[:, :],
                            in1=F_sb[:, :], s0=_CH_S0, s1=_CH_S1,
                        )
                    return em

                # ---------- EDSCW branch (phases ed, mx) ----------
                def ed_branch():
                    with tc.high_priority(offset=25):
                        s_ps = psum.tile([P, n], f32, tag="ps_s", name="ps_s",
                                         bufs=1)
                        for i in range(4):
                            nc.tensor.matmul(s_ps[:, :], ident[:, :],
                                             x4t[:, i, :],
                                             start=(i == 0), stop=(i == 3))
                        # inva = 1/(s/4 + 1e-12) fused, straight from PSUM
                        inva = T("inva", bufs=3)
                        load["dve"] += n * 1.04 + 250
                        nc.vector._custom_dve(
                            ravg_op, out=inva[:, :], in0=s_ps[:, :],
                            in1=epsc[:, :], s0=_CH_S0, s1=_CH_S1, imm2=0.25,
                        )
                        r_all = T4("r_all", bufs=2)
                        inva_b = inva[:, :].unsqueeze(1).broadcast_to(
                            [P, 4, n])
                        load["dve"] += 4 * n * 0.52 + 120
                        nc.vector.tensor_mul(r_all[:, :, :], x4t[:, :, :],
                                             inva_b)
                        dsc_all = T4("dsc_all", bufs=2)
                        load["dve"] += 4 * n * 1.04 + 120
                        nc.vector._custom_dve(
                            dsc_op, out=dsc_all[:, :, :], in0=r_all[:, :, :],
                            s0=_CH_S0, s1=_CH_S1,
                        )
                    e_all = T4("e_all", bufs=3)
                    nc.scalar.activation(e_all[:, :, :], dsc_all[:, :, :],
                                         Exp, scale=2.0)
                    pe_all = T4("pe_all", bufs=3)
                    tap_mul(pe_all, e_all, x4t)
                    E_ps = tree(e_all, "ps_A", bufs=2)
                    Pn_ps = tree(pe_all, "ps_B", bufs=3)
                    E_sb = T("E_sb", bufs=3)
                    ed = T("ed", bufs=3)
                    with tc.high_priority(offset=15):
                        nc.scalar.copy(E_sb[:, :], E_ps[:, :])
                        load["dve"] += n * 1.04 + 250
                        nc.vector._custom_dve(
                            div_op, out=ed[:, :], in0=Pn_ps[:, :],
                            in1=E_sb[:, :], s0=_CH_S0, s1=_CH_S1,
                        )
                    return ed

                if ph == "em":
                    em = em_branch()
                    nc.sync.dma_start(out=out_d[:, sl], in_=em[:, :])
                elif ph == "ed":
                    ed = ed_branch()
                    nc.sync.dma_start(out=out_d[:, sl], in_=ed[:, :])
                else:
                    # f-exp is ready as soon as x4t lands: emit EM first so
                    # Act/Pool start early; the EDSCW head chain is pulled
                    # forward by its priorities anyway
                    em = em_branch()
                    ed = ed_branch()
                    bb = T("bb", bufs=2)
                    nc.sync.dma_start(out=bb[:, :], in_=betam_d[:, mo:mo + n])
                    # out = em + bb*(ed - em)
                    dif = T("dif", bufs=3)
                    win_op(lambda: nc.vector.tensor_sub(
                               dif[:, :], ed[:, :], em[:, :]),
                           lambda: nc.gpsimd.tensor_sub(
                               dif[:, :], ed[:, :], em[:, :]), n)
                    bd = T("bd", bufs=3)
                    win_op(lambda: nc.vector.tensor_mul(
                               bd[:, :], dif[:, :], bb[:, :]),
                           lambda: nc.gpsimd.tensor_mul(
                               bd[:, :], dif[:, :], bb[:, :]), n)
                    ot = T("ot", bufs=3)
                    win_op(lambda: nc.vector.tensor_add(
                               ot[:, :], em[:, :], bd[:, :]),
                           lambda: nc.gpsimd.tensor_add(
                               ot[:, :], em[:, :], bd[:, :]), n)
                    nc.sync.dma_start(out=out_d[:, sl], in_=ot[:, :])
                    mo += n
    nc.finalize()
    return nc


def _get_nc(n0, n1, nm):
    key = (n0, n1, nm)
    if key not in _COMPILED:
        _COMPILED[key] = _build(*key)
    return _COMPILED[key]


def _shard_inputs(x, beta):
    """Host-side: classify window positions by beta, permute windows into
    [beta==0 | beta==1 | mixed] order, split taps, pack [P,4,NWIN] bf16."""
    import ml_dtypes

    bfl = ml_dtypes.bfloat16
    x = np.ascontiguousarray(x, dtype=np.float32)
    beta = np.asarray(beta, dtype=np.float32).reshape(NWIN)

    cls0 = beta <= 0.0
    cls1 = beta >= 1.0
    clsm = ~(cls0 | cls1)
    perm = np.concatenate(
        [np.nonzero(cls0)[0], np.nonzero(cls1)[0], np.nonzero(clsm)[0]])
    n0, n1 = int(cls0.sum()), int(cls1.sum())
    nm = NWIN - n0 - n1
    inv = np.empty(NWIN, dtype=np.int64)
    inv[perm] = np.arange(NWIN)

    bm = beta[perm[n0 + n1:]].astype(bfl) if nm else None
    ident = np.ascontiguousarray(np.eye(P, dtype=bfl))

    in_maps = []
    for core in range(NCORES):
        planes = x[core * BPC:(core + 1) * BPC].reshape(P, H, W)
        v = planes.reshape(P, OH, 2, OW, 2)
        x4 = np.empty((P, 4, NWIN), dtype=bfl)
        x4[:, 0, :] = v[:, :, 0, :, 0].reshape(P, NWIN)[:, perm]
        x4[:, 1, :] = v[:, :, 0, :, 1].reshape(P, NWIN)[:, perm]
        x4[:, 2, :] = v[:, :, 1, :, 0].reshape(P, NWIN)[:, perm]
        x4[:, 3, :] = v[:, :, 1, :, 1].reshape(P, NWIN)[:, perm]
        m = {"x4": x4, "ident": ident}
        if nm:
            m["betam"] = np.ascontiguousarray(
                np.broadcast_to(bm.reshape(1, nm), (P, nm)))
        in_maps.append(m)
    return in_maps, (n0, n1, nm), inv


LAST = {}


def kernel(x, beta, trace=False, trace_kwargs=None):
    from concourse.bass_utils import run_bass_kernel_spmd

    in_maps, (n0, n1, nm), inv = _shard_inputs(np.asarray(x),
                                               np.asarray(beta))
    nc = _get_nc(n0, n1, nm)
    res = run_bass_kernel_spmd(
        nc, in_maps, core_ids=list(range(NCORES)),
        trace=trace, **(trace_kwargs or {}),
    )
    LAST["exec_time_ns"] = getattr(res, "exec_time_ns", None)
    LAST["results"] = res
    LAST["nc"] = nc
    out = np.empty((B, C, OH, OW), dtype=np.float32)
    for core in range(NCORES):
        o = np.asarray(res.results[core]["out"]).astype(np.float32)
        out[core * BPC:(core + 1) * BPC] = o[:, inv].reshape(BPC, C, OH, OW)
    return out
